# revision 9
# baseline (speedup 1.0000x reference)
"""CPRLinear Trainium2 kernel.

y = x[:, col_indices] @ W_deq.T + bias, where W_deq is the per-128-column-tile
affine dequantization of [W_high_q | W_low_q] (int32 codes).

Sharding: out_features (8192) split across 8 NeuronCores, 1024 rows each.
x / col_indices replicated (x is shipped transposed so the column permutation
becomes a contiguous row gather on device).

Per-core device pipeline:
  - indirect-DMA row-gather of xT by col_indices -> x̃T (k-major), DVE cast to bf16
  - Wq streamed in k-chunks: HWDGE load (int32, natural [o,k] layout),
    DVE dual-scalar tensor_scalar dequant (q - z) * s with per-partition
    scale/zero columns, xbar DMA-transpose to k-major bf16
  - TensorE: y[b,o] accumulated over 64 k-tiles in 4 PSUM groups
    (2 b-blocks x 2 o-halves, N=512), bias folded in via a ones-row matmul
  - ACT evacuates PSUM -> SBUF, HWDGE stores y slab [256, 1024] f32
"""

import os
import sys

import numpy as np

for _p in ("/root/.axon_site", "/root/.axon_site/_ro/trn_rl_repo",
           "/root/.axon_site/_ro/pypackages", "/opt/trn_rl_repo"):
    if os.path.isdir(_p) and _p not in sys.path:
        sys.path.append(_p)

B, IN, OUT = 256, 8192, 8192
N_CORES = 8
O_SLAB = OUT // N_CORES          # 1024 out rows per core
N_HIGH, N_LOW = 2048, 6144
TILE = 128
NT = IN // TILE                  # 64 k-tiles
K_CHUNK = 1024                   # k elements processed per chunk
N_CHUNKS = IN // K_CHUNK         # 8
TPC = K_CHUNK // TILE            # 8 k-tiles per chunk
OB = O_SLAB // TILE              # 8 o-blocks per core
N_HIGH_CHUNKS = N_HIGH // K_CHUNK  # 2

_PROGRAM = None


def _build_program(n_bodies=1):
    import concourse.bass as bass
    import concourse.bacc as bacc
    import concourse.tile as tile
    import concourse.mybir as mybir

    f32 = mybir.dt.float32
    bf16 = mybir.dt.bfloat16
    i32 = mybir.dt.int32

    nc = bacc.Bacc(
        "TRN2",
        target_bir_lowering=False,
        debug=False,
        enable_asserts=False,
        num_devices=N_CORES,
    )

    xT = nc.dram_tensor("xT", [IN, B], f32, kind="ExternalInput").ap()
    ci = nc.dram_tensor("ci", [128, NT], i32, kind="ExternalInput").ap()
    whq = nc.dram_tensor("whq", [O_SLAB, N_HIGH], i32, kind="ExternalInput").ap()
    wlq = nc.dram_tensor("wlq", [O_SLAB, N_LOW], i32, kind="ExternalInput").ap()
    sT = nc.dram_tensor("sT", [OB, 128, NT], f32, kind="ExternalInput").ap()
    zT = nc.dram_tensor("zT", [OB, 128, NT], f32, kind="ExternalInput").ap()
    bias = nc.dram_tensor("bias", [1, O_SLAB], f32, kind="ExternalInput").ap()
    y = nc.dram_tensor("y", [B, O_SLAB], f32, kind="ExternalOutput").ap()

    with tile.TileContext(nc) as tc:
        for _ in range(n_bodies):
            _kernel_body(tc, xT, ci, whq, wlq, sT, zT, bias, y,
                         bass=bass, mybir=mybir, tile=tile)

    nc.compile()
    return nc


def _kernel_body(tc, xT, ci, whq, wlq, sT, zT, bias, y, *, bass, mybir, tile):
    from contextlib import ExitStack

    nc = tc.nc
    f32 = mybir.dt.float32
    bf16 = mybir.dt.bfloat16
    i32 = mybir.dt.int32
    Alu = mybir.AluOpType

    with ExitStack() as ctx:
        const = ctx.enter_context(tc.tile_pool(name="const", bufs=1))
        xstage = ctx.enter_context(tc.tile_pool(name="xstage", bufs=3))
        wqpool = ctx.enter_context(tc.tile_pool(name="wq", bufs=8))
        wnpool = ctx.enter_context(tc.tile_pool(name="wn", bufs=8))
        wtpool = ctx.enter_context(tc.tile_pool(name="wt", bufs=3))
        ypool = ctx.enter_context(tc.tile_pool(name="yout", bufs=4))
        psum = ctx.enter_context(tc.tile_pool(name="psum", bufs=1, space="PSUM"))

        # --- constants ---
        ci_sb = const.tile([128, NT], i32, tag="ci")
        nc.sync.dma_start(out=ci_sb, in_=ci)

        sT_sb = const.tile([128, OB, NT], f32, tag="sT")
        zT_sb = const.tile([128, OB, NT], f32, tag="zT")
        for ob in range(OB):
            nc.sync.dma_start(out=sT_sb[:, ob, :], in_=sT[ob])
            nc.sync.dma_start(out=zT_sb[:, ob, :], in_=zT[ob])

        ones = const.tile([128, 128], bf16, tag="ones")
        nc.vector.memset(ones, 1.0)

        wbias = const.tile([128, O_SLAB], bf16, tag="wbias")
        nc.vector.memset(wbias, 0.0)
        bias_f = const.tile([1, O_SLAB], f32, tag="biasf")
        nc.sync.dma_start(out=bias_f, in_=bias)
        nc.vector.tensor_copy(wbias[0:1, :], bias_f)

        # x̃T bf16 chunks, persistent across the whole kernel
        xb = [const.tile([128, TPC, B], bf16, tag=f"xb{c}", name=f"xb{c}")
              for c in range(N_CHUNKS)]

        # PSUM accumulation groups: [b-block][o-half]
        ps = [[psum.tile([128, 512], f32, tag=f"ps{bb}{oc}", name=f"ps{bb}{oc}")
               for oc in range(2)] for bb in range(2)]

        def w_src(c, ob):
            # natural-layout Wq rows [128 o, K_CHUNK k] for chunk c, o-block ob
            if c < N_HIGH_CHUNKS:
                return whq[ob * 128:(ob + 1) * 128,
                           c * K_CHUNK:(c + 1) * K_CHUNK]
            cl = c - N_HIGH_CHUNKS
            return wlq[ob * 128:(ob + 1) * 128,
                       cl * K_CHUNK:(cl + 1) * K_CHUNK]

        for c in range(N_CHUNKS):
            # ---- x path: gather 8 k-tiles of xT rows, cast to bf16 ----
            xf = xstage.tile([128, TPC, B], f32, tag="xf")
            for t in range(TPC):
                kt = c * TPC + t
                nc.gpsimd.indirect_dma_start(
                    out=xf[:, t, :],
                    out_offset=None,
                    in_=xT,
                    in_offset=bass.IndirectOffsetOnAxis(
                        ap=ci_sb[:, kt:kt + 1], axis=0),
                )
            nc.vector.tensor_copy(xb[c][:, :, :], xf[:, :, :])

            # ---- W path: load int32, dequant, transpose to k-major ----
            # wt layout: [k-in-tile 128, ob 8, t 8, o-in-block 128]
            wt = wtpool.tile([128, OB, TPC, 128], bf16, tag="wt")
            for ob in range(OB):
                # SWDGE cast-DMA: int32 codes (0..63) arrive in SBUF as bf16
                # exactly; halves the SBUF write traffic for W
                wq = wqpool.tile([128, K_CHUNK], bf16, tag="wq")
                nc.gpsimd.dma_start(out=wq, in_=w_src(c, ob))
                wn = wnpool.tile([128, K_CHUNK], bf16, tag="wn")
                for t in range(TPC):
                    kt = c * TPC + t
                    nc.vector.tensor_scalar(
                        out=wn[:, t * 128:(t + 1) * 128],
                        in0=wq[:, t * 128:(t + 1) * 128],
                        scalar1=zT_sb[:, ob, kt:kt + 1],
                        scalar2=sT_sb[:, ob, kt:kt + 1],
                        op0=Alu.subtract,
                        op1=Alu.mult,
                    )
                # contiguous destination block for the xbar transpose
                nc.scalar.dma_start_transpose(wt[:, ob, :, :], wn)

            # ---- matmuls: accumulate y over this chunk's k-tiles ----
            for t in range(TPC):
                kt = c * TPC + t
                for bb in range(2):
                    lhsT = xb[c][:, t, bb * 128:(bb + 1) * 128]
                    for oc in range(2):
                        rhs = wt[:, oc * 4:(oc + 1) * 4, t, :]
                        nc.tensor.matmul(
                            ps[bb][oc][:, :],
                            lhsT,
                            rhs,
                            start=(kt == 0),
                            stop=False,
                        )

        # ---- bias via ones-row matmul, closes each accumulation group ----
        for bb in range(2):
            for oc in range(2):
                nc.tensor.matmul(
                    ps[bb][oc][:, :],
                    ones,
                    wbias[:, oc * 512:(oc + 1) * 512],
                    start=False,
                    stop=True,
                )

        # ---- evacuate PSUM and store ----
        for bb in range(2):
            for oc in range(2):
                ysb = ypool.tile([128, 512], f32, tag="ysb")
                nc.scalar.copy(ysb, ps[bb][oc][:, :])
                nc.sync.dma_start(
                    out=y[bb * 128:(bb + 1) * 128, oc * 512:(oc + 1) * 512],
                    in_=ysb,
                )


def get_program():
    global _PROGRAM
    if _PROGRAM is None:
        _PROGRAM = _build_program()
    return _PROGRAM


def make_in_maps(x, W_high_q, W_low_q, scales_high, zeros_high,
                 scales_low, zeros_low, bias, col_indices):
    """Host-side sharding / layout prep. Returns per-core input dicts."""
    x = np.asarray(x)
    xT = np.ascontiguousarray(x.T.astype(np.float32, copy=False))  # [IN, B]
    ci = np.ascontiguousarray(
        np.asarray(col_indices).astype(np.int32, copy=False).reshape(NT, 128).T
    )  # [128, NT]; ci[p, t] = col_indices[t*128 + p]

    s_all = np.concatenate(
        [np.asarray(scales_high, dtype=np.float32),
         np.asarray(scales_low, dtype=np.float32)], axis=0)   # [NT, OUT]
    z_all = np.concatenate(
        [np.asarray(zeros_high, dtype=np.float32),
         np.asarray(zeros_low, dtype=np.float32)], axis=0)    # [NT, OUT]
    sT_full = np.ascontiguousarray(s_all.T)                   # [OUT, NT]
    zT_full = np.ascontiguousarray(z_all.T)                   # [OUT, NT]

    whq = np.ascontiguousarray(np.asarray(W_high_q, dtype=np.int32))
    wlq = np.ascontiguousarray(np.asarray(W_low_q, dtype=np.int32))
    bias = np.asarray(bias, dtype=np.float32)

    in_maps = []
    for c in range(N_CORES):
        sl = slice(c * O_SLAB, (c + 1) * O_SLAB)
        in_maps.append({
            "xT": xT,
            "ci": ci,
            "whq": np.ascontiguousarray(whq[sl]),
            "wlq": np.ascontiguousarray(wlq[sl]),
            "sT": np.ascontiguousarray(sT_full[sl].reshape(OB, 128, NT)),
            "zT": np.ascontiguousarray(zT_full[sl].reshape(OB, 128, NT)),
            "bias": np.ascontiguousarray(bias[sl].reshape(1, O_SLAB)),
        })
    return in_maps


def run_on_device(in_maps):
    from concourse.bass_utils import run_bass_kernel_spmd
    nc = get_program()
    res = run_bass_kernel_spmd(nc, in_maps, list(range(N_CORES)))
    out = np.concatenate(
        [res.results[c]["y"] for c in range(N_CORES)], axis=1)
    return np.ascontiguousarray(out.astype(np.float32, copy=False))


def kernel(x, W_high_q, W_low_q, scales_high, zeros_high,
           scales_low, zeros_low, bias, col_indices):
    in_maps = make_in_maps(x, W_high_q, W_low_q, scales_high, zeros_high,
                           scales_low, zeros_low, bias, col_indices)
    return run_on_device(in_maps)


# ---------------------------------------------------------------------------
# Benchmark path (test.py only): inputs parked on-device, jit built once,
# dispatches pipelined so the axon-tunnel round trip amortizes away.
# ---------------------------------------------------------------------------

class DeviceRunner:
    def __init__(self, in_maps, nc=None):
        import jax
        import numpy as _np
        from jax.experimental.shard_map import shard_map
        from jax.sharding import Mesh, NamedSharding, PartitionSpec
        import concourse.mybir as mybir
        from concourse.bass2jax import (
            _bass_exec_p, install_neuronx_cc_hook, partition_id_tensor)

        install_neuronx_cc_hook()
        if nc is None:
            nc = get_program()
        partition_name = (nc.partition_id_tensor.name
                          if nc.partition_id_tensor else None)

        in_names, out_names, out_avals, zero_outs = [], [], [], []
        for alloc in nc.m.functions[0].allocations:
            if not isinstance(alloc, mybir.MemoryLocationSet):
                continue
            name = alloc.memorylocations[0].name
            if alloc.kind == "ExternalInput":
                if name != partition_name:
                    in_names.append(name)
            elif alloc.kind == "ExternalOutput":
                shape = tuple(alloc.tensor_shape)
                dtype = mybir.dt.np(alloc.dtype)
                out_names.append(name)
                out_avals.append(jax.core.ShapedArray(shape, dtype))
                zero_outs.append(_np.zeros(shape, dtype))
        n_params = len(in_names)
        all_in_names = list(in_names) + list(out_names)
        if partition_name is not None:
            all_in_names.append(partition_name)

        def _body(*args):
            operands = list(args)
            if partition_name is not None:
                operands.append(partition_id_tensor())
            return tuple(_bass_exec_p.bind(
                *operands,
                out_avals=tuple(out_avals),
                in_names=tuple(all_in_names),
                out_names=tuple(out_names),
                lowering_input_output_aliases=(),
                sim_require_finite=True,
                sim_require_nnan=True,
                nc=nc,
            ))

        devices = jax.devices()[:N_CORES]
        mesh = Mesh(_np.asarray(devices), ("core",))
        spec = PartitionSpec("core")
        nin = n_params + len(zero_outs)
        self.fn = jax.jit(
            shard_map(_body, mesh=mesh,
                      in_specs=(spec,) * nin,
                      out_specs=(spec,) * len(out_names),
                      check_rep=False),
            keep_unused=True,
        )
        sharding = NamedSharding(mesh, spec)
        concat_in = [
            _np.concatenate([in_maps[c][k] for c in range(N_CORES)], axis=0)
            for k in in_names
        ]
        concat_zeros = [
            _np.zeros((N_CORES * z.shape[0], *z.shape[1:]), z.dtype)
            for z in zero_outs
        ]
        self.args = [jax.device_put(a, sharding)
                     for a in concat_in + concat_zeros]
        self.out_names = out_names
        self.out_avals = out_avals
        self._jax = jax

    def run(self):
        return self.fn(*self.args)

    def fetch(self, outs):
        import numpy as _np
        y = _np.asarray(outs[self.out_names.index("y")])
        y = y.reshape(N_CORES, B, O_SLAB)
        return _np.concatenate(list(y), axis=1)

    def bench(self, iters=20):
        import time
        jax = self._jax
        # warm
        outs = self.run()
        jax.block_until_ready(outs)
        t0 = time.perf_counter()
        last = None
        for _ in range(iters):
            last = self.run()
        jax.block_until_ready(last)
        dt = (time.perf_counter() - t0) / iters
        return dt, self.fetch(last)
